# revision 1
# baseline (speedup 1.0000x reference)
"""Dual-key additive attention (nn_Attention_58059367908033) on 8 trn2 NeuronCores.

Reference computation (per batch b, head h, with n = 64*64 = 4096 positions,
d = 128, scale = d**-0.5):
    q  = Wq_h  @ fmap[b]          # [d, n]   (channels-major, "n" = spatial)
    k1 = Wk1_h @ fmap[b]          # [d, n]
    v  = Wv_h  @ fmap[b]          # [d, n]
    k2 = Wk2_h @ x[b]             # [d, n]
    sim  = (scale * q)^T (k1+k2)  # [n, n]  (q rows, key cols)
    attn = softmax(sim, axis=-1)
    out[b, h*d:(h+1)*d] = (attn @ v^T)^T  # [d, n] -> reshape [d, 64, 64]

Sharding: 8 cores = (b in 2) x (h in 2) x (key-half kh in 2).  Each core
computes, for its (b, h) and its 2048-key slice, the *unnormalized*
    U[d, q]   = sum_{k in slice} exp(scale * sim[k, q]) * vT[k, d]
    D[1, q]   = sum_{k in slice} exp(scale * sim[k, q])
streamed flash-attention style (no max subtraction: |scale*sim| is O(1) for
these inputs, fp32 exp is exact-safe).  The host adds the two key-half
partials and divides -- mathematically exact softmax-attention.

On-chip layout is fully transposed (keys on partitions for exp, contraction
over d for QK^T and over k for PV), so no transposes are needed anywhere and
U comes out channels-major, matching the output layout directly.
"""

import ml_dtypes
import numpy as np

BF16_NP = ml_dtypes.bfloat16

import concourse.bass as bass
import concourse.mybir as mybir
import concourse.tile as tile
from concourse import bacc
from concourse.bass_utils import run_bass_kernel_spmd

HEADS = 2
D = 128          # dim head
C1 = 256         # fmap channels
C2 = 2048        # x channels
N = 4096         # spatial positions (64*64) = queries = keys
KSL = 2048       # keys per core (half)
SCALE = float(D) ** -0.5

F32 = mybir.dt.float32
BF16 = mybir.dt.bfloat16

# key-chunk = 512 keys (4 k-tiles of 128); query-chunk = 512 queries
KC = 4           # key chunks per core
KT = 4           # k-tiles (128) per key chunk
QC = 8           # query chunks of 512
QW = 512

_COMPILED = {}


def _build_program():
    nc = bacc.Bacc("TRN2", target_bir_lowering=False, debug=False, num_devices=8)

    # ---- DRAM parameters (per-core data, same program on all 8 cores) ----
    d_fmap = nc.dram_tensor("fmap_b", [2, 128, N], BF16, kind="ExternalInput").ap()
    d_fmapk = nc.dram_tensor("fmap_k", [2, 128, KSL], BF16, kind="ExternalInput").ap()
    d_xs = nc.dram_tensor("xs", [16, 128, KSL], BF16, kind="ExternalInput").ap()
    d_wqT = nc.dram_tensor("wqT", [2, 128, D], BF16, kind="ExternalInput").ap()
    d_wk1T = nc.dram_tensor("wk1T", [2, 128, D], BF16, kind="ExternalInput").ap()
    d_wvT = nc.dram_tensor("wvT", [2, 128, D], BF16, kind="ExternalInput").ap()
    d_wk2T = nc.dram_tensor("wk2T", [16, 128, D], BF16, kind="ExternalInput").ap()
    d_outU = nc.dram_tensor("outU", [128, N], F32, kind="ExternalOutput").ap()
    d_den = nc.dram_tensor("denom", [1, N], F32, kind="ExternalOutput").ap()

    with tile.TileContext(nc) as tc:
        with (
            tc.tile_pool(name="wts", bufs=1) as wts,
            tc.tile_pool(name="fm", bufs=1) as fm,
            tc.tile_pool(name="big", bufs=1) as big,
            tc.tile_pool(name="xs", bufs=2) as xsp,
            tc.tile_pool(name="ex", bufs=8) as exp_pool,
            tc.tile_pool(name="acc", bufs=2) as accp,
            tc.tile_pool(name="st", bufs=2) as st,
            tc.tile_pool(name="ps_s", bufs=3, space="PSUM") as ps_s,
            tc.tile_pool(name="ps_o", bufs=1, space="PSUM") as ps_o,
            tc.tile_pool(name="ps_d", bufs=1, space="PSUM") as ps_d,
        ):
            # ---- input DMAs split across the two DGE streams ----
            # gpsimd stream: weights -> fmap_k -> fmap + x half-B chunks
            # sync stream:   x half-A chunks (+ outputs at the end)
            # Each stream executes its transfers serially (~200 GB/s), so the
            # 8.4 MB of x is split between both.
            # prologue spread over 4 DGE streams (tensor/vector queues are
            # idle until real compute starts, so borrowing them is free)
            fmapk = fm.tile([128, 2, KSL], BF16, tag="fmapk")
            nc.gpsimd.dma_start(fmapk[:], d_fmapk.rearrange("t p n -> p t n"))
            wq = wts.tile([128, 2, D], BF16, tag="wq")
            wk1 = wts.tile([128, 2, D], BF16, tag="wk1")
            wv = wts.tile([128, 2, D], BF16, tag="wv")
            nc.gpsimd.dma_start(wv[:], d_wvT.rearrange("t p d -> p t d"))
            nc.gpsimd.dma_start(wk1[:], d_wk1T.rearrange("t p d -> p t d"))
            wk2 = wts.tile([128, 16, D], BF16, tag="wk2")
            nc.gpsimd.dma_start(wk2[:], d_wk2T.rearrange("t p d -> p t d"))
            nc.gpsimd.dma_start(wq[:], d_wqT.rearrange("t p d -> p t d"))

            ones = wts.tile([128, 1], BF16, tag="ones")
            nc.vector.memset(ones[:], 1.0)

            xv = d_xs.rearrange("t p n -> p t n")
            x_tiles = [xsp.tile([128, 16, QW], BF16, tag="x", name=f"x{i}")
                       for i in range(2)]

            def load_x_half(kc, half, eng):
                xt = x_tiles[kc % 2]
                hs = slice(half * 8, half * 8 + 8)
                eng.dma_start(xt[:, hs, :],
                              xv[:, hs, kc * QW:(kc + 1) * QW])

            fmap = fm.tile([128, 2, N], BF16, tag="fmap")
            fv = d_fmap.rearrange("t p n -> p t n")

            def load_fmap(fc, eng):
                fs = slice(fc * 1024, (fc + 1) * 1024)
                eng.dma_start(fmap[:, :, fs], fv[:, :, fs])

            load_x_half(0, 0, nc.sync)
            load_x_half(0, 1, nc.scalar)
            load_fmap(0, nc.gpsimd)
            load_x_half(1, 0, nc.sync)
            load_fmap(1, nc.gpsimd)
            load_x_half(1, 1, nc.gpsimd)

            # ---- HAM warm-up: dummy matmuls (no data deps) fill the PE's
            # DMA-wait idle windows during the sparse prologue phase so the
            # clock gate opens early and stays open.
            warm = wts.tile([128, 512], BF16, tag="warm")
            nc.vector.memset(warm[:], 0.0)

            def warmup(n):
                for _ in range(n):
                    wps = ps_s.tile([128, QW], F32, tag="ps_sim", name="pswarm")
                    nc.tensor.matmul(wps[:], warm[:, :128], warm[:],
                                     start=True, stop=True)

            warmup(26)

            # ---- vT tiles [k=128, d] via fmap_k-stationary matmuls ----
            vT = big.tile([128, 16, D], BF16, tag="vT")
            for kt in range(16):
                ps = ps_s.tile([128, D], F32, tag="ps_sim", name="psv")
                ksl = slice(kt * 128, (kt + 1) * 128)
                nc.tensor.matmul(ps[:], fmapk[:, 0, ksl], wv[:, 0, :], start=True, stop=False)
                nc.tensor.matmul(ps[:], fmapk[:, 1, ksl], wv[:, 1, :], start=False, stop=True)
                nc.scalar.copy(vT[:, kt, :], ps[:])

            # ---- q projection, one 512-wide chunk at a time (woven in) ----
            q_sb = big.tile([128, N], BF16, tag="q")

            def build_q(nch):
                ps = ps_s.tile([128, QW], F32, tag="ps_sim", name="psq")
                sl = slice(nch * QW, (nch + 1) * QW)
                nc.tensor.matmul(ps[:], wq[:, 0, :], fmap[:, 0, sl], start=True, stop=False)
                nc.tensor.matmul(ps[:], wq[:, 1, :], fmap[:, 1, sl], start=False, stop=True)
                nc.vector.tensor_copy(q_sb[:, sl], ps[:])

            warmup(6)
            build_q(0)
            build_q(1)
            warmup(6)

            ksum = big.tile([128, KSL], BF16, tag="ksum")

            # ---- attention: qc outer; ksum built lazily during qc==0 ----
            # Per qc: outU accumulates all 16 PV matmuls in one PSUM bank;
            # denominator = bf16 running sum of exp tiles on DVE + one final
            # ones-matmul.  One-stage software pipeline keeps PE dense.
            from collections import deque
            pendq = deque()
            for qc in range(QC):
                qsl = slice(qc * QW, (qc + 1) * QW)
                ops = ps_o.tile([128, QW], F32, tag="ps_out", name=f"ops{qc}")
                acc_a = accp.tile([128, QW], BF16, tag="dacca", name=f"acca{qc}")
                acc_b = accp.tile([128, QW], BF16, tag="daccb", name=f"accb{qc}")
                for kc in range(KC):
                    if qc == 0:
                        # build ksum[:, kc] = Wk1 @ fmap_k + Wk2 @ xs
                        kps = ps_d.tile([128, QW], F32, tag="ps_den", name="kps")
                        sl = slice(kc * QW, (kc + 1) * QW)
                        nc.tensor.matmul(kps[:], wk1[:, 0, :], fmapk[:, 0, sl],
                                         start=True, stop=False)
                        nc.tensor.matmul(kps[:], wk1[:, 1, :], fmapk[:, 1, sl],
                                         start=False, stop=False)
                        xt = x_tiles[kc % 2]
                        for ct in range(16):
                            nc.tensor.matmul(kps[:], wk2[:, ct, :], xt[:, ct, :],
                                             start=False, stop=(ct == 15))
                        nc.vector.tensor_copy(ksum[:, sl], kps[:])
                        if kc + 2 < KC:
                            load_x_half(kc + 2, 0, nc.sync)
                            load_x_half(kc + 2, 1, nc.sync)
                        if kc == KC - 1:
                            load_fmap(2, nc.gpsimd)
                            load_fmap(3, nc.gpsimd)
                    if kc == 1 and qc + 2 < QC:
                        build_q(qc + 2)
                    for sg in range(KT // 2):
                        sps = ps_s.tile([128, 2, QW], F32, tag="ps_sim")
                        et = exp_pool.tile([128, 2, QW], BF16, tag="exp")
                        for j in range(2):
                            kk = kc * KT + sg * 2 + j
                            nc.tensor.matmul(
                                sps[:, j, :],
                                ksum[:, kk * 128:(kk + 1) * 128], q_sb[:, qsl],
                                start=True, stop=True)
                        nc.scalar.activation(et[:], sps[:],
                                             mybir.ActivationFunctionType.Exp,
                                             scale=SCALE)
                        if len(pendq) >= 1:
                            pendq.popleft()()

                        first = (kc == 0 and sg == 0)
                        last = (kc == KC - 1 and sg == KT // 2 - 1)

                        def _pend(qc0=qc, kc0=kc, sg0=sg, et0=et, ops0=ops,
                                  acc0=acc_a, acc1=acc_b, first=first, last=last):
                            for j in range(2):
                                kk = kc0 * KT + sg0 * 2 + j
                                nc.tensor.matmul(ops0[:], vT[:, kk, :],
                                                 et0[:, j, :],
                                                 start=(first and j == 0),
                                                 stop=(last and j == 1))
                            # denominator running sums on DVE (bf16): two
                            # independent accumulators so the adds pipeline
                            # instead of forming one serial RAW chain
                            if first:
                                nc.vector.tensor_copy(acc0[:], et0[:, 0, :])
                                nc.vector.tensor_copy(acc1[:], et0[:, 1, :])
                            else:
                                nc.vector.tensor_add(acc0[:], acc0[:], et0[:, 0, :])
                                nc.vector.tensor_add(acc1[:], acc1[:], et0[:, 1, :])
                            if last:
                                qsl0 = slice(qc0 * QW, (qc0 + 1) * QW)
                                dps = ps_d.tile([1, QW], F32, tag="ps_den")
                                nc.tensor.matmul(dps[:], ones[:], acc0[:],
                                                 start=True, stop=False)
                                nc.tensor.matmul(dps[:], ones[:], acc1[:],
                                                 start=False, stop=True)
                                den_st = st.tile([1, QW], F32, tag="den_st")
                                nc.vector.tensor_copy(den_st[:], dps[:])
                                nc.sync.dma_start(d_den[:, qsl0], den_st[:])
                                out_st = st.tile([128, QW], F32, tag="out_st")
                                nc.vector.tensor_copy(out_st[:], ops0[:])
                                nc.sync.dma_start(d_outU[:, qsl0], out_st[:])
                        pendq.append(_pend)
            while pendq:
                pendq.popleft()()

    nc.compile()
    return nc


def _prep_inputs(fmap, x, Wqkv, Wk2):
    """Host-side slicing: per-core input dicts. Core c = b*4 + h*2 + kh."""
    fmap = np.ascontiguousarray(fmap, dtype=np.float32)
    x = np.ascontiguousarray(x, dtype=np.float32)
    Wqkv = np.ascontiguousarray(Wqkv, dtype=np.float32)
    Wk2 = np.ascontiguousarray(Wk2, dtype=np.float32)

    in_maps = []
    for c in range(8):
        b, h, kh = c // 4, (c // 2) % 2, c % 2
        fb = fmap[b].reshape(C1, N)
        xb = x[b].reshape(C2, N)
        ks = slice(kh * KSL, (kh + 1) * KSL)
        wq = Wqkv[h * D:(h + 1) * D]              # [128, 256]
        wk1 = Wqkv[C1 + h * D:C1 + (h + 1) * D]
        wv = Wqkv[2 * C1 + h * D:2 * C1 + (h + 1) * D]
        wk2 = Wk2[h * D:(h + 1) * D]              # [128, 2048]
        in_maps.append({
            "fmap_b": fb.reshape(2, 128, N).astype(BF16_NP),
            "fmap_k": np.ascontiguousarray(fb[:, ks].reshape(2, 128, KSL)).astype(BF16_NP),
            "xs": np.ascontiguousarray(xb[:, ks].reshape(16, 128, KSL)).astype(BF16_NP),
            "wqT": np.ascontiguousarray(wq.T).reshape(2, 128, D).astype(BF16_NP),
            "wk1T": np.ascontiguousarray(wk1.T).reshape(2, 128, D).astype(BF16_NP),
            "wvT": np.ascontiguousarray(wv.T).reshape(2, 128, D).astype(BF16_NP),
            "wk2T": np.ascontiguousarray(wk2.T).reshape(16, 128, D).astype(BF16_NP),
        })
    return in_maps


def _combine(results):
    """Host epilogue: add key-half partials, normalize, assemble output."""
    out = np.empty((2, HEADS * D, 64, 64), dtype=np.float32)
    for b in range(2):
        for h in range(2):
            c0 = b * 4 + h * 2
            U = results[c0]["outU"] + results[c0 + 1]["outU"]     # [128, N]
            Dn = results[c0]["denom"] + results[c0 + 1]["denom"]  # [1, N]
            out[b, h * D:(h + 1) * D] = (U / Dn).reshape(D, 64, 64)
    return out


def run_on_device(in_maps, trace=False, **kw):
    if "nc" not in _COMPILED:
        _COMPILED["nc"] = _build_program()
    return run_bass_kernel_spmd(_COMPILED["nc"], in_maps, list(range(8)),
                                trace=trace, **kw)


def kernel(fmap, x, Wqkv, Wk2):
    in_maps = _prep_inputs(fmap, x, Wqkv, Wk2)
    res = run_on_device(in_maps)
    return _combine(res.results)

